# revision 53
# baseline (speedup 1.0000x reference)
"""Trainium2 Bass kernel for nn_Attention_5643587027315 (8 NeuronCores).

Sharding: tensor-parallel over the 16 heads (2 heads per core).
Per core: qkv^T projection (d-major) -> RoPE -> flash-style attention with
kerple bias applied as a multiplicative table (1+a*d)^(-p) -> AllToAll of the
context (token resharding) -> token-sharded output projection.

v3 over the original baseline:
- x is transposed on the HOST; x^T loads are plain per-(dk, chunk) DMAs
  (no device DMA-transpose), ordered so chunk 0 and the kerple/rope tables
  land first.
- one merged kerple sliding window per head ([128, 2, W3_LEN], 2 DMAs,
  ~2 MB instead of 6 MB); kview slices it with a reversed free stride.
- vtok transpose results land via one strided DVE copy per tile.
- per-group normalization: ONE [65,1024] f32 ACT copy moves numerator and
  denominator off PSUM together; the den row is re-staged to partition 0
  by a DVE copy (the custom reciprocal op cannot read PSUM or shifted
  partition bases on HW). Boundary copies stay on ACT -- putting them on
  DVE stalls the exp->kerple-mul consumer chain.
- the final (hl=1, qc2=1) group normalizes via a PE ones-broadcast matmul
  + DVE multiply (pool chain is off the critical tail then).
- per-(qc2, b) A2A staging DMAs; merged out-proj lhs loads; out-proj
  weight load issued during attention; y copies alternate ACT/DVE.
"""
import numpy as np
import ml_dtypes

B, S, DM, H, D = 2, 2048, 1024, 16, 64
N = B * S            # 4096 flattened tokens
SCALE = 1.0 / float(np.sqrt(D))
QKSCALE = SCALE / 64.0       # k weights are scaled x64 for fp8 range
bf16 = ml_dtypes.bfloat16
fp8 = ml_dtypes.float8_e4m3fn

KT_LEN = 4224        # kerple table length per head (pad)
W3_LEN = 3970        # sliding-window columns held in SBUF per head

_GRAPH_CACHE = {}


def _build_graph():
    import concourse.bass as bass
    import concourse.mybir as mybir
    import concourse.tile as tile
    from concourse import bacc
    from concourse.masks import make_identity
    from contextlib import ExitStack

    F32 = mybir.dt.float32
    BF16 = mybir.dt.bfloat16
    FP8 = mybir.dt.float8e4
    DR = mybir.MatmulPerfMode.DoubleRow

    nc = bacc.Bacc("TRN2", target_bir_lowering=False, num_devices=8)

    x_p = nc.declare_dram_parameter("xT_bf", [DM, N], BF16, isOutput=False)
    wT_p = nc.declare_dram_parameter("wT", [DM, 384], BF16, isOutput=False)
    ow_p = nc.declare_dram_parameter("out_wT", [DM, DM], BF16, isOutput=False)
    cc_p = nc.declare_dram_parameter("cc", [128, N], BF16, isOutput=False)
    ss_p = nc.declare_dram_parameter("ssg", [128, N], BF16, isOutput=False)
    kt_p = nc.declare_dram_parameter("ktab", [2, KT_LEN], BF16, isOutput=False)
    out_p = nc.declare_dram_parameter("out", [512, DM], F32, isOutput=True)

    Exp = mybir.ActivationFunctionType.Exp

    with tile.TileContext(nc) as tc, ExitStack() as top:
        const = top.enter_context(tc.tile_pool(name="const", bufs=1))
        qkv_sb = top.enter_context(tc.tile_pool(name="qkv_sb", bufs=1))
        ctxp = top.enter_context(tc.tile_pool(name="ctxp", bufs=1))
        dram = top.enter_context(tc.tile_pool(name="dram", bufs=1, space="DRAM"))

        w_sb = const.tile([128, 8, 384], BF16)
        cc_sb = const.tile([128, N], BF16)
        ss_sb = const.tile([128, N], BF16)
        ident = const.tile([128, 128], BF16)
        ones64 = const.tile([1, 64], BF16)
        vt_c = [const.tile([128, 8, 130], BF16, name=f"vt{i}")
                for i in range(4)]
        # kerple sliding windows: w3[p, hl, xx] = T[hl][127 + p + xx]
        w3_sb = const.tile([128, 2, W3_LEN], BF16)

        make_identity(nc, ident)
        nc.vector.memset(ones64, 1.0)
        nc.sync.dma_start(out=w_sb, in_=wT_p.rearrange("(a p) r -> p a r", p=128))
        ow_pool = top.enter_context(tc.tile_pool(name="ow", bufs=1))
        ow_sb = ow_pool.tile([128, 8, DM], BF16)

        q_c = [qkv_sb.tile([128, 1024], BF16, name=f"qc{i}") for i in range(4)]
        k_c = [qkv_sb.tile([128, 1024], BF16, name=f"kc{i}") for i in range(4)]
        v_c = [qkv_sb.tile([128, 1024], BF16, name=f"vc{i}") for i in range(4)]

        with ExitStack() as ph_a:
            xt_pool = ph_a.enter_context(tc.tile_pool(name="xt", bufs=1))
            qk_psum = ph_a.enter_context(
                tc.tile_pool(name="qk_psum", bufs=6, space="PSUM"))
            tp_psum = ph_a.enter_context(
                tc.tile_pool(name="tp_psum", bufs=2, space="PSUM"))
            rope_pool = ph_a.enter_context(tc.tile_pool(name="rope", bufs=1))

            # x^T loads split per (chunk, dk); DMA queue order favors the
            # critical path: ch0 first, then rope/kerple tables, then the rest.
            xt = {}

            def load_chunk(ch):
                cs = slice(ch * 1024, (ch + 1) * 1024)
                for dk in range(8):
                    t = xt_pool.tile([128, 1024], BF16,
                                     name=f"xt{dk}_{ch}", tag="xt", bufs=32)
                    nc.sync.dma_start(
                        out=t, in_=x_p[dk * 128:(dk + 1) * 128, cs])
                    xt[(dk, ch)] = t

            load_chunk(0)
            nc.sync.dma_start(out=cc_sb, in_=cc_p[:])
            nc.sync.dma_start(out=ss_sb, in_=ss_p[:])
            for hl in range(2):
                src = bass.AP(tensor=kt_p, offset=hl * KT_LEN + 127,
                              ap=[[1, 128], [1, W3_LEN]])
                nc.sync.dma_start(out=w3_sb[:, hl, :], in_=src)
            for ch in (2, 1, 3):
                load_chunk(ch)

            dsts = (q_c, k_c, v_c)

            def rope_one(X, ch, pool, bufs):
                cs = slice(ch * 1024, (ch + 1) * 1024)
                xsw = pool.tile([128, 1024], BF16, name="xsw",
                                tag="xsw", bufs=bufs)
                for (d0, s0) in ((0, 32), (32, 0), (64, 96), (96, 64)):
                    nc.sync.dma_start(
                        out=xsw[d0:d0 + 32, :], in_=X[s0:s0 + 32, :])
                a_t = pool.tile([128, 1024], BF16, name="ropea",
                                tag="ropea", bufs=bufs)
                nc.vector.tensor_mul(a_t, X, cc_sb[:, cs])
                nc.vector.tensor_mul(xsw, xsw, ss_sb[:, cs])
                nc.vector.tensor_add(X, a_t, xsw)

            def rope_chunk(ch, pool, bufs, eng):
                rope_one(q_c[ch], ch, pool, bufs)
                rope_one(k_c[ch], ch, pool, bufs)

            def vtok_chunk(ch, psum_pool, psum_tag):
                nc.vector.memset(vt_c[ch][:, :, 64], 1.0)
                nc.vector.memset(vt_c[ch][:, :, 129], 1.0)
                for tj in range(8):
                    pst = psum_pool.tile([128, 128], BF16, name="tps",
                                         tag=psum_tag)
                    nc.tensor.transpose(
                        pst, v_c[ch][:, tj * 128:tj * 128 + 128], ident)
                    # one strided copy: cols (0:64) and (65:129) of slot tj
                    dst = vt_c[ch][:, tj, :].rearrange(
                        "p (a q) -> p a q", a=2)[:, :, 0:64]
                    nc.vector.tensor_copy(dst, pst.rearrange(
                        "p (a q) -> p a q", a=2))

            def qkv_chunk(ch, copy_eng):
                pss = [qk_psum.tile([128, 512], F32, name=f"qkps{t2}{r3}",
                                    tag="qkps")
                       for t2 in range(2) for r3 in range(3)]
                for dk in range(8):
                    for r3 in range(3):
                        for t2 in range(2):
                            nc.tensor.matmul(
                                pss[t2 * 3 + r3],
                                w_sb[:, dk, r3 * 128:(r3 + 1) * 128],
                                xt[(dk, ch)][:, t2 * 512:t2 * 512 + 512],
                                start=(dk == 0), stop=(dk == 7))
                for t2 in range(2):
                    for r3 in range(3):
                        dst = dsts[r3][ch][:, t2 * 512:t2 * 512 + 512]
                        if r3 == 2:
                            nc.vector.tensor_copy(dst, pss[t2 * 3 + r3])
                        else:
                            copy_eng(dst, pss[t2 * 3 + r3])

            # chunks 0,2,1 fully in phase A; chunk 3 rope+vtok deferred
            for ch in (0, 2, 1):
                qkv_chunk(ch, nc.scalar.copy)
                rope_chunk(ch, rope_pool, 2, nc.vector)
                vtok_chunk(ch, tp_psum, "tps")
            qkv_chunk(3, nc.vector.tensor_copy)

        # out-proj weights: load during attention (DMA is idle then)
        nc.sync.dma_start(out=ow_sb, in_=ow_p.rearrange("(a p) r -> p a r", p=128))

        # ---- attention (with fused per-chunk normalization) ----
        ctxn = [ctxp.tile([64, N], BF16, name=f"ctxn{i}") for i in range(2)]
        a2a_in = [dram.tile([8, 64, 512], BF16, name=f"a2ai{i}")
                  for i in range(2)]
        a2a_out = [dram.tile([8, 64, 512], BF16, name=f"a2ao{i}")
                   for i in range(2)]
        with ExitStack() as ph_b:
            sc_psum = ph_b.enter_context(
                tc.tile_pool(name="sc_psum", bufs=2, space="PSUM"))
            ctx_psum = ph_b.enter_context(
                tc.tile_pool(name="ctx_psum", bufs=2, space="PSUM"))
            e_pool = ph_b.enter_context(tc.tile_pool(name="e_pool", bufs=4))
            div_pool = ph_b.enter_context(tc.tile_pool(name="div", bufs=2))

            rope2 = ph_b.enter_context(tc.tile_pool(name="rope2", bufs=1))

            def deferred_wave(ch, r3, dst):
                # qkv wave for a deferred chunk, borrowing a scores psum slot
                ps = sc_psum.tile([128, 1024], F32, name="dfps", tag="scps")
                for dk in range(8):
                    for t2 in range(2):
                        nc.tensor.matmul(
                            ps[:, t2 * 512:(t2 + 1) * 512],
                            w_sb[:, dk, r3 * 128:(r3 + 1) * 128],
                            xt[(dk, ch)][:, t2 * 512:t2 * 512 + 512],
                            start=(dk == 0), stop=(dk == 7))
                nc.vector.tensor_copy(dst, ps)

            for hl in range(2):
                for qc2 in range(2):
                    h64 = hl * 64
                    acc = [ctx_psum.tile([65, 1024], F32, name=f"acc{b}",
                                         tag="acc") for b in range(B)]
                    for kt_i in range(16):
                        if hl == 0 and qc2 == 0 and kt_i == 2:
                            rope_chunk(3, rope2, 1, nc.vector)
                            vtok_chunk(3, sc_psum, "scps")
                        k0 = kt_i * 128
                        # kerple window slice (reversed free stride):
                        # kview[p, j] = T[2175 + (k0 + p) - (1024*qc2 + j)]
                        xx0 = k0 - 1024 * qc2 + 2048
                        kview = w3_sb[:, hl, xx0:xx0 - 1024:-1]
                        for b in range(B):
                            ps = sc_psum.tile([128, 1024], F32, name="scps",
                                              tag="scps")
                            kc = k_c[2 * b + (k0 >= 1024)]
                            kcol = k0 % 1024
                            qc = q_c[2 * b + qc2]
                            for half in range(2):
                                nc.tensor.matmul(
                                    ps[:, half * 512:(half + 1) * 512],
                                    kc[h64:h64 + 64, kcol:kcol + 128],
                                    qc[h64:h64 + 64,
                                       half * 512:(half + 1) * 512],
                                    start=True, stop=True)
                            e = e_pool.tile([128, 1024], BF16, name="e", tag="e")
                            nc.scalar.activation(e, ps, Exp, scale=SCALE)
                            nc.vector.tensor_mul(e, e, kview)
                            gti = b * 16 + kt_i
                            vtt = vt_c[gti // 8][:, gti % 8,
                                                 hl * 65:hl * 65 + 65]
                            for half in range(2):
                                nc.tensor.matmul(
                                    acc[b][:, half * 512:(half + 1) * 512],
                                    vtt,
                                    e[:, half * 512:(half + 1) * 512],
                                    start=(kt_i == 0), stop=(kt_i == 15))
                    last_grp = (hl == 1 and qc2 == 1)
                    for b in range(B):
                        sl = slice(b * S + qc2 * 1024, b * S + qc2 * 1024 + 1024)
                        # one copy moves numerator AND denominator off PSUM
                        acc65 = div_pool.tile([65, 1024], F32, name="acc65",
                                              tag="acc65", bufs=4)
                        nc.scalar.copy(acc65, acc[b])
                        acc_sb = acc65[0:64, :]
                        den_t = div_pool.tile([1, 1024], F32, name="dent",
                                              tag="dent", bufs=2)
                        nc.vector.tensor_copy(den_t, acc65[64:65, :])
                        rcp = div_pool.tile([1, 1024], F32, name="rcp", tag="rcp")
                        nc.vector.reciprocal_approx_fast(rcp, den_t)
                        if last_grp:
                            # attention matmuls are done: spread 1/den across
                            # partitions on the idle PE, multiply on DVE
                            rcp_b = div_pool.tile([1, 1024], BF16, name="rcpb",
                                                  tag="rcpb", bufs=2)
                            nc.vector.tensor_copy(rcp_b, rcp)
                            rb_ps = sc_psum.tile([64, 1024], F32, name="scps",
                                                 tag="scps")
                            for half in range(2):
                                nc.tensor.matmul(
                                    rb_ps[:, half * 512:(half + 1) * 512],
                                    ones64,
                                    rcp_b[:, half * 512:(half + 1) * 512],
                                    start=True, stop=True)
                            nc.vector.tensor_mul(ctxn[hl][:, sl], acc_sb, rb_ps)
                        else:
                            rb_sb = div_pool.tile([64, 1024], F32, name="rbsb",
                                                  tag="rbsb")
                            nc.gpsimd.partition_broadcast(rb_sb, rcp)
                            nc.gpsimd.tensor_mul(ctxn[hl][:, sl], acc_sb, rb_sb)
                        # stage this token block into the A2A input eagerly
                        j0 = 4 * b + 2 * qc2
                        nc.sync.dma_start(
                            out=a2a_in[hl][j0:j0 + 2].transpose([1, 0, 2]),
                            in_=ctxn[hl][:, sl].rearrange(
                                "p (j q) -> p j q", j=2))
                nc.gpsimd.collective_compute(
                    "AllToAll", mybir.AluOpType.bypass,
                    replica_groups=[list(range(8))],
                    ins=[a2a_in[hl].opt()], outs=[a2a_out[hl].opt()])

        # ---- output projection for this core's 512-token slice ----
        with ExitStack() as ph_y:
            y_psum = ph_y.enter_context(
                tc.tile_pool(name="y_psum", bufs=2, space="PSUM"))
            y_lhs = ph_y.enter_context(tc.tile_pool(name="y_lhs", bufs=1))
            y_out = ph_y.enter_context(tc.tile_pool(name="y_out", bufs=2))
            lh_all = y_lhs.tile([128, 8, 512], BF16)
            for hl in range(2):
                src = bass.AP(tensor=a2a_out[hl].tensor,
                              offset=a2a_out[hl].offset,
                              ap=[[512, 64], [64 * 512, 8], [1, 512]])
                nc.sync.dma_start(
                    out=lh_all[hl * 64:hl * 64 + 64, :, :], in_=src)
            for tt in range(4):
                ps_y = y_psum.tile([128, 1024], F32, name="psy", tag="psy")
                for j in range(8):
                    for dmc in range(2):
                        nc.tensor.matmul(
                            ps_y[:, dmc * 512:(dmc + 1) * 512],
                            lh_all[:, j, tt * 128:(tt + 1) * 128],
                            ow_sb[:, j, dmc * 512:(dmc + 1) * 512],
                            start=(j == 0), stop=(j == 7))
                y_sb = y_out.tile([128, 1024], F32, name="ysb", tag="ysb")
                if tt % 2 == 0:
                    nc.scalar.copy(y_sb, ps_y)
                else:
                    nc.vector.tensor_copy(y_sb, ps_y)
                nc.sync.dma_start(
                    out=out_p[tt * 128:(tt + 1) * 128, :], in_=y_sb)

    nc.compile()
    return nc


def _host_prep(x, qkv_w, out_w, bias_p, bias_a, rope_freqs, c):
    x = np.asarray(x, np.float32).reshape(N, DM)
    qkv_w = np.asarray(qkv_w, np.float32)
    out_w = np.asarray(out_w, np.float32)
    bias_p = np.asarray(bias_p, np.float32).reshape(H)
    bias_a = np.asarray(bias_a, np.float32).reshape(H)
    freqs = np.asarray(rope_freqs, np.float32)

    h0, h1 = 2 * c, 2 * c + 1
    rows = []
    for blk in range(3):
        for h in (h0, h1):
            rows.append(qkv_w[blk * 1024 + h * 64: blk * 1024 + h * 64 + 64])
    wT = np.ascontiguousarray(np.concatenate(rows, 0).T)

    pos = np.arange(S, dtype=np.float32)
    ang = pos[:, None] * freqs[None, :]
    cosT = np.cos(ang).T
    sinT = np.sin(ang).T
    cc64 = np.concatenate([cosT, cosT], 0)
    ss64 = np.concatenate([-sinT, sinT], 0)
    CC = np.tile(np.concatenate([cc64, cc64], 0), (1, B))
    SSg = np.tile(np.concatenate([ss64, ss64], 0), (1, B))

    p = np.maximum(bias_p, 0.01)
    a = np.maximum(bias_a, 0.01)
    # T[hl][m] = (1 + a*|m - 2175|)^(-p); kview[p, j] = T[255 - p + yy0 + j]
    idx = np.abs(np.arange(KT_LEN, dtype=np.float32) - 2175.0)
    ktab = np.stack([(1.0 + a[h] * idx) ** (-p[h]) for h in (h0, h1)], 0)

    xT = np.ascontiguousarray(x.T)
    return {
        "xT_bf": xT.astype(bf16),
        "wT": wT.astype(bf16),
        "out_wT": np.ascontiguousarray(out_w.T).astype(bf16),
        "cc": np.ascontiguousarray(CC).astype(bf16),
        "ssg": np.ascontiguousarray(SSg).astype(bf16),
        "ktab": np.ascontiguousarray(ktab).astype(bf16),
    }


def kernel(x, qkv_w, out_w, bias_p, bias_a, rope_freqs, _trace=False):
    from concourse.bass_utils import run_bass_kernel_spmd

    if "nc" not in _GRAPH_CACHE:
        _GRAPH_CACHE["nc"] = _build_graph()
    nc = _GRAPH_CACHE["nc"]

    in_maps = [
        _host_prep(x, qkv_w, out_w, bias_p, bias_a, rope_freqs, c)
        for c in range(8)
    ]
    res = run_bass_kernel_spmd(nc, in_maps, core_ids=list(range(8)),
                               trace=_trace)
    _GRAPH_CACHE["last_result"] = res
    y = np.concatenate([np.asarray(res.results[c]["out"]) for c in range(8)], 0)
    return np.ascontiguousarray(y.reshape(B, S, DM)).astype(np.float32)


# revision 58
# speedup vs baseline: 1.4854x; 1.4854x over previous
"""Trainium2 Bass kernel for nn_Attention_5643587027315 (8 NeuronCores).

Sharding: tensor-parallel over the 16 heads (2 heads per core).
Per core: qkv^T projection (d-major) -> RoPE -> flash-style attention with
kerple bias applied as a multiplicative table (1+a*d)^(-p) -> AllToAll of the
context (token resharding) -> token-sharded output projection.

v3 over the original baseline:
- x is transposed on the HOST; x^T loads are plain per-(dk, chunk) DMAs
  (no device DMA-transpose), ordered so chunk 0 and the kerple/rope tables
  land first.
- one merged kerple sliding window per head ([128, 2, W3_LEN], 2 DMAs,
  ~2 MB instead of 6 MB); kview slices it with a reversed free stride.
- vtok transpose results land via one strided DVE copy per tile.
- per-group normalization: ONE [65,1024] f32 ACT copy moves numerator and
  denominator off PSUM together; the den row is re-staged to partition 0
  by a DVE copy (the custom reciprocal op cannot read PSUM or shifted
  partition bases on HW). Boundary copies stay on ACT -- putting them on
  DVE stalls the exp->kerple-mul consumer chain.
- the final (hl=1, qc2=1) group normalizes via a PE ones-broadcast matmul
  + DVE multiply (pool chain is off the critical tail then).
- per-(qc2, b) A2A staging DMAs; merged out-proj lhs loads; out-proj
  weight load issued during attention; y copies alternate ACT/DVE.
"""
import numpy as np
import ml_dtypes

B, S, DM, H, D = 2, 2048, 1024, 16, 64
N = B * S            # 4096 flattened tokens
SCALE = 1.0 / float(np.sqrt(D))
QKSCALE = SCALE / 64.0       # k weights are scaled x64 for fp8 range
bf16 = ml_dtypes.bfloat16
fp8 = ml_dtypes.float8_e4m3fn

KT_LEN = 4224        # kerple table length per head (pad)
W3_LEN = 3970        # sliding-window columns held in SBUF per head

_GRAPH_CACHE = {}


def _build_graph():
    import concourse.bass as bass
    import concourse.mybir as mybir
    import concourse.tile as tile
    from concourse import bacc
    from concourse.masks import make_identity
    from contextlib import ExitStack

    F32 = mybir.dt.float32
    BF16 = mybir.dt.bfloat16
    FP8 = mybir.dt.float8e4
    DR = mybir.MatmulPerfMode.DoubleRow

    nc = bacc.Bacc("TRN2", target_bir_lowering=False, num_devices=8)

    x_p = nc.declare_dram_parameter("xT_bf", [DM, N], BF16, isOutput=False)
    wT_p = nc.declare_dram_parameter("wT", [DM, 384], BF16, isOutput=False)
    ow_p = nc.declare_dram_parameter("out_wT", [DM, DM], BF16, isOutput=False)
    cc_p = nc.declare_dram_parameter("cc", [128, N], BF16, isOutput=False)
    ss_p = nc.declare_dram_parameter("ssg", [128, N], BF16, isOutput=False)
    kt_p = nc.declare_dram_parameter("ktab", [2, KT_LEN], BF16, isOutput=False)
    out_p = nc.declare_dram_parameter("out", [512, DM], BF16, isOutput=True)

    Exp = mybir.ActivationFunctionType.Exp

    with tile.TileContext(nc) as tc, ExitStack() as top:
        const = top.enter_context(tc.tile_pool(name="const", bufs=1))
        qkv_sb = top.enter_context(tc.tile_pool(name="qkv_sb", bufs=1))
        ctxp = top.enter_context(tc.tile_pool(name="ctxp", bufs=1))
        dram = top.enter_context(tc.tile_pool(name="dram", bufs=1, space="DRAM"))

        w_sb = const.tile([128, 8, 384], BF16)
        cc_sb = const.tile([128, N], BF16)
        ss_sb = const.tile([128, N], BF16)
        ident = const.tile([128, 128], BF16)
        ones64 = const.tile([1, 64], BF16)
        vt_c = [const.tile([128, 8, 130], BF16, name=f"vt{i}")
                for i in range(4)]
        # kerple sliding windows: w3[p, hl, xx] = T[hl][127 + p + xx]
        w3_sb = const.tile([128, 2, W3_LEN], BF16)

        make_identity(nc, ident)
        nc.vector.memset(ones64, 1.0)
        nc.sync.dma_start(out=w_sb, in_=wT_p.rearrange("(a p) r -> p a r", p=128))
        ow_pool = top.enter_context(tc.tile_pool(name="ow", bufs=1))
        ow_sb = ow_pool.tile([128, 8, DM], BF16)

        q_c = [qkv_sb.tile([128, 1024], BF16, name=f"qc{i}") for i in range(4)]
        k_c = [qkv_sb.tile([128, 1024], BF16, name=f"kc{i}") for i in range(4)]
        v_c = [qkv_sb.tile([128, 1024], BF16, name=f"vc{i}") for i in range(4)]

        with ExitStack() as ph_a:
            xt_pool = ph_a.enter_context(tc.tile_pool(name="xt", bufs=1))
            qk_psum = ph_a.enter_context(
                tc.tile_pool(name="qk_psum", bufs=6, space="PSUM"))
            tp_psum = ph_a.enter_context(
                tc.tile_pool(name="tp_psum", bufs=2, space="PSUM"))
            rope_pool = ph_a.enter_context(tc.tile_pool(name="rope", bufs=1))

            # x^T loads split per (chunk, dk); DMA queue order favors the
            # critical path: ch0 first, then rope/kerple tables, then the rest.
            xt = {}

            def load_chunk(ch):
                cs = slice(ch * 1024, (ch + 1) * 1024)
                for dk in range(8):
                    t = xt_pool.tile([128, 1024], BF16,
                                     name=f"xt{dk}_{ch}", tag="xt", bufs=32)
                    nc.sync.dma_start(
                        out=t, in_=x_p[dk * 128:(dk + 1) * 128, cs])
                    xt[(dk, ch)] = t

            load_chunk(0)
            nc.sync.dma_start(out=cc_sb, in_=cc_p[:])
            nc.sync.dma_start(out=ss_sb, in_=ss_p[:])
            for hl in range(2):
                src = bass.AP(tensor=kt_p, offset=hl * KT_LEN + 127,
                              ap=[[1, 128], [1, W3_LEN]])
                nc.sync.dma_start(out=w3_sb[:, hl, :], in_=src)
            for ch in (2, 1, 3):
                load_chunk(ch)

            dsts = (q_c, k_c, v_c)

            def rope_one(X, ch, pool, bufs):
                cs = slice(ch * 1024, (ch + 1) * 1024)
                xsw = pool.tile([128, 1024], BF16, name="xsw",
                                tag="xsw", bufs=bufs)
                for (d0, s0) in ((0, 32), (32, 0), (64, 96), (96, 64)):
                    nc.sync.dma_start(
                        out=xsw[d0:d0 + 32, :], in_=X[s0:s0 + 32, :])
                a_t = pool.tile([128, 1024], BF16, name="ropea",
                                tag="ropea", bufs=bufs)
                nc.vector.tensor_mul(a_t, X, cc_sb[:, cs])
                nc.vector.tensor_mul(xsw, xsw, ss_sb[:, cs])
                nc.vector.tensor_add(X, a_t, xsw)

            def rope_chunk(ch, pool, bufs, eng):
                rope_one(q_c[ch], ch, pool, bufs)
                rope_one(k_c[ch], ch, pool, bufs)

            def vtok_chunk(ch, psum_pool, psum_tag):
                nc.vector.memset(vt_c[ch][:, :, 64], 1.0)
                nc.vector.memset(vt_c[ch][:, :, 129], 1.0)
                for tj in range(8):
                    pst = psum_pool.tile([128, 128], BF16, name="tps",
                                         tag=psum_tag)
                    nc.tensor.transpose(
                        pst, v_c[ch][:, tj * 128:tj * 128 + 128], ident)
                    # one strided copy: cols (0:64) and (65:129) of slot tj
                    dst = vt_c[ch][:, tj, :].rearrange(
                        "p (a q) -> p a q", a=2)[:, :, 0:64]
                    nc.vector.tensor_copy(dst, pst.rearrange(
                        "p (a q) -> p a q", a=2))

            def qkv_chunk(ch, copy_eng):
                pss = [qk_psum.tile([128, 512], F32, name=f"qkps{t2}{r3}",
                                    tag="qkps")
                       for t2 in range(2) for r3 in range(3)]
                for dk in range(8):
                    for r3 in range(3):
                        for t2 in range(2):
                            nc.tensor.matmul(
                                pss[t2 * 3 + r3],
                                w_sb[:, dk, r3 * 128:(r3 + 1) * 128],
                                xt[(dk, ch)][:, t2 * 512:t2 * 512 + 512],
                                start=(dk == 0), stop=(dk == 7))
                for t2 in range(2):
                    for r3 in range(3):
                        dst = dsts[r3][ch][:, t2 * 512:t2 * 512 + 512]
                        if r3 == 2:
                            nc.vector.tensor_copy(dst, pss[t2 * 3 + r3])
                        else:
                            copy_eng(dst, pss[t2 * 3 + r3])

            # chunks 0,2,1 fully in phase A; chunk 3 rope+vtok deferred
            for ch in (0, 2, 1):
                qkv_chunk(ch, nc.scalar.copy)
                rope_chunk(ch, rope_pool, 2, nc.vector)
                vtok_chunk(ch, tp_psum, "tps")
            qkv_chunk(3, nc.vector.tensor_copy)

        # out-proj weights: load during attention (DMA is idle then)
        nc.sync.dma_start(out=ow_sb, in_=ow_p.rearrange("(a p) r -> p a r", p=128))

        # ---- attention (with fused per-chunk normalization) ----
        ctxn = [ctxp.tile([64, N], BF16, name=f"ctxn{i}") for i in range(2)]
        a2a_in = [dram.tile([8, 64, 512], BF16, name=f"a2ai{i}")
                  for i in range(2)]
        a2a_out = [dram.tile([8, 64, 512], BF16, name=f"a2ao{i}")
                   for i in range(2)]
        with ExitStack() as ph_b:
            sc_psum = ph_b.enter_context(
                tc.tile_pool(name="sc_psum", bufs=2, space="PSUM"))
            ctx_psum = ph_b.enter_context(
                tc.tile_pool(name="ctx_psum", bufs=2, space="PSUM"))
            e_pool = ph_b.enter_context(tc.tile_pool(name="e_pool", bufs=4))
            div_pool = ph_b.enter_context(tc.tile_pool(name="div", bufs=2))

            rope2 = ph_b.enter_context(tc.tile_pool(name="rope2", bufs=1))

            def deferred_wave(ch, r3, dst):
                # qkv wave for a deferred chunk, borrowing a scores psum slot
                ps = sc_psum.tile([128, 1024], F32, name="dfps", tag="scps")
                for dk in range(8):
                    for t2 in range(2):
                        nc.tensor.matmul(
                            ps[:, t2 * 512:(t2 + 1) * 512],
                            w_sb[:, dk, r3 * 128:(r3 + 1) * 128],
                            xt[(dk, ch)][:, t2 * 512:t2 * 512 + 512],
                            start=(dk == 0), stop=(dk == 7))
                nc.vector.tensor_copy(dst, ps)

            for hl in range(2):
                for qc2 in range(2):
                    h64 = hl * 64
                    acc = [ctx_psum.tile([65, 1024], F32, name=f"acc{b}",
                                         tag="acc") for b in range(B)]
                    for kt_i in range(16):
                        if hl == 0 and qc2 == 0 and kt_i == 2:
                            rope_chunk(3, rope2, 1, nc.vector)
                            vtok_chunk(3, sc_psum, "scps")
                        k0 = kt_i * 128
                        # kerple window slice (reversed free stride):
                        # kview[p, j] = T[2175 + (k0 + p) - (1024*qc2 + j)]
                        xx0 = k0 - 1024 * qc2 + 2048
                        kview = w3_sb[:, hl, xx0:xx0 - 1024:-1]
                        for b in range(B):
                            ps = sc_psum.tile([128, 1024], F32, name="scps",
                                              tag="scps")
                            kc = k_c[2 * b + (k0 >= 1024)]
                            kcol = k0 % 1024
                            qc = q_c[2 * b + qc2]
                            for half in range(2):
                                nc.tensor.matmul(
                                    ps[:, half * 512:(half + 1) * 512],
                                    kc[h64:h64 + 64, kcol:kcol + 128],
                                    qc[h64:h64 + 64,
                                       half * 512:(half + 1) * 512],
                                    start=True, stop=True)
                            e = e_pool.tile([128, 1024], BF16, name="e", tag="e")
                            nc.scalar.activation(e, ps, Exp, scale=SCALE)
                            nc.vector.tensor_mul(e, e, kview)
                            gti = b * 16 + kt_i
                            vtt = vt_c[gti // 8][:, gti % 8,
                                                 hl * 65:hl * 65 + 65]
                            for half in range(2):
                                nc.tensor.matmul(
                                    acc[b][:, half * 512:(half + 1) * 512],
                                    vtt,
                                    e[:, half * 512:(half + 1) * 512],
                                    start=(kt_i == 0), stop=(kt_i == 15))
                    last_grp = (hl == 1 and qc2 == 1)
                    for b in range(B):
                        sl = slice(b * S + qc2 * 1024, b * S + qc2 * 1024 + 1024)
                        # one copy moves numerator AND denominator off PSUM
                        acc65 = div_pool.tile([65, 1024], F32, name="acc65",
                                              tag="acc65", bufs=4)
                        nc.scalar.copy(acc65, acc[b])
                        acc_sb = acc65[0:64, :]
                        den_t = div_pool.tile([1, 1024], F32, name="dent",
                                              tag="dent", bufs=2)
                        nc.vector.tensor_copy(den_t, acc65[64:65, :])
                        rcp = div_pool.tile([1, 1024], F32, name="rcp", tag="rcp")
                        nc.vector.reciprocal_approx_fast(rcp, den_t)
                        if last_grp:
                            # attention matmuls are done: spread 1/den across
                            # partitions on the idle PE, multiply on DVE
                            rcp_b = div_pool.tile([1, 1024], BF16, name="rcpb",
                                                  tag="rcpb", bufs=2)
                            nc.vector.tensor_copy(rcp_b, rcp)
                            rb_ps = sc_psum.tile([64, 1024], F32, name="scps",
                                                 tag="scps")
                            for half in range(2):
                                nc.tensor.matmul(
                                    rb_ps[:, half * 512:(half + 1) * 512],
                                    ones64,
                                    rcp_b[:, half * 512:(half + 1) * 512],
                                    start=True, stop=True)
                            nc.vector.tensor_mul(ctxn[hl][:, sl], acc_sb, rb_ps)
                        else:
                            rb_sb = div_pool.tile([64, 1024], F32, name="rbsb",
                                                  tag="rbsb")
                            nc.gpsimd.partition_broadcast(rb_sb, rcp)
                            nc.gpsimd.tensor_mul(ctxn[hl][:, sl], acc_sb, rb_sb)
                        # stage this token block into the A2A input eagerly
                        j0 = 4 * b + 2 * qc2
                        nc.sync.dma_start(
                            out=a2a_in[hl][j0:j0 + 2].transpose([1, 0, 2]),
                            in_=ctxn[hl][:, sl].rearrange(
                                "p (j q) -> p j q", j=2))
                nc.gpsimd.collective_compute(
                    "AllToAll", mybir.AluOpType.bypass,
                    replica_groups=[list(range(8))],
                    ins=[a2a_in[hl].opt()], outs=[a2a_out[hl].opt()])

        # ---- output projection for this core's 512-token slice ----
        with ExitStack() as ph_y:
            y_psum = ph_y.enter_context(
                tc.tile_pool(name="y_psum", bufs=2, space="PSUM"))
            y_lhs = ph_y.enter_context(tc.tile_pool(name="y_lhs", bufs=1))
            y_out = ph_y.enter_context(tc.tile_pool(name="y_out", bufs=2))
            lh_all = y_lhs.tile([128, 8, 512], BF16)
            for hl in range(2):
                src = bass.AP(tensor=a2a_out[hl].tensor,
                              offset=a2a_out[hl].offset,
                              ap=[[512, 64], [64 * 512, 8], [1, 512]])
                nc.sync.dma_start(
                    out=lh_all[hl * 64:hl * 64 + 64, :, :], in_=src)
            for tt in range(4):
                ps_y = y_psum.tile([128, 1024], F32, name="psy", tag="psy")
                for j in range(8):
                    for dmc in range(2):
                        nc.tensor.matmul(
                            ps_y[:, dmc * 512:(dmc + 1) * 512],
                            lh_all[:, j, tt * 128:(tt + 1) * 128],
                            ow_sb[:, j, dmc * 512:(dmc + 1) * 512],
                            start=(j == 0), stop=(j == 7))
                y_sb = y_out.tile([128, 1024], BF16, name="ysb", tag="ysb")
                if tt % 2 == 0:
                    nc.scalar.copy(y_sb, ps_y)
                else:
                    nc.vector.tensor_copy(y_sb, ps_y)
                nc.sync.dma_start(
                    out=out_p[tt * 128:(tt + 1) * 128, :], in_=y_sb)

    nc.compile()
    return nc


def _host_prep(x, qkv_w, out_w, bias_p, bias_a, rope_freqs, c):
    x = np.asarray(x, np.float32).reshape(N, DM)
    qkv_w = np.asarray(qkv_w, np.float32)
    out_w = np.asarray(out_w, np.float32)
    bias_p = np.asarray(bias_p, np.float32).reshape(H)
    bias_a = np.asarray(bias_a, np.float32).reshape(H)
    freqs = np.asarray(rope_freqs, np.float32)

    h0, h1 = 2 * c, 2 * c + 1
    rows = []
    for blk in range(3):
        for h in (h0, h1):
            rows.append(qkv_w[blk * 1024 + h * 64: blk * 1024 + h * 64 + 64])
    wT = np.ascontiguousarray(np.concatenate(rows, 0).T)

    pos = np.arange(S, dtype=np.float32)
    ang = pos[:, None] * freqs[None, :]
    cosT = np.cos(ang).T
    sinT = np.sin(ang).T
    cc64 = np.concatenate([cosT, cosT], 0)
    ss64 = np.concatenate([-sinT, sinT], 0)
    CC = np.tile(np.concatenate([cc64, cc64], 0), (1, B))
    SSg = np.tile(np.concatenate([ss64, ss64], 0), (1, B))

    p = np.maximum(bias_p, 0.01)
    a = np.maximum(bias_a, 0.01)
    # T[hl][m] = (1 + a*|m - 2175|)^(-p); kview[p, j] = T[255 - p + yy0 + j]
    idx = np.abs(np.arange(KT_LEN, dtype=np.float32) - 2175.0)
    ktab = np.stack([(1.0 + a[h] * idx) ** (-p[h]) for h in (h0, h1)], 0)

    xT = np.ascontiguousarray(x.T)
    return {
        "xT_bf": xT.astype(bf16),
        "wT": wT.astype(bf16),
        "out_wT": np.ascontiguousarray(out_w.T).astype(bf16),
        "cc": np.ascontiguousarray(CC).astype(bf16),
        "ssg": np.ascontiguousarray(SSg).astype(bf16),
        "ktab": np.ascontiguousarray(ktab).astype(bf16),
    }


def kernel(x, qkv_w, out_w, bias_p, bias_a, rope_freqs, _trace=False):
    from concourse.bass_utils import run_bass_kernel_spmd

    if "nc" not in _GRAPH_CACHE:
        _GRAPH_CACHE["nc"] = _build_graph()
    nc = _GRAPH_CACHE["nc"]

    in_maps = [
        _host_prep(x, qkv_w, out_w, bias_p, bias_a, rope_freqs, c)
        for c in range(8)
    ]
    res = run_bass_kernel_spmd(nc, in_maps, core_ids=list(range(8)),
                               trace=_trace)
    _GRAPH_CACHE["last_result"] = res
    y = np.concatenate([np.asarray(res.results[c]["out"]) for c in range(8)], 0)
    return np.ascontiguousarray(y.reshape(B, S, DM)).astype(np.float32)
